# revision 3
# baseline (speedup 1.0000x reference)
"""Trainium2 Bass kernel for DigitConvolutionalModel forward pass.

Model: x[B,784] -> 3x3 valid conv (single channel) -> flatten[676]
       -> relu(.@W1+b1) -> relu(.@W2+b2) -> .@W3+b3 -> [B,10]

Strategy:
  - Pure data parallel: batch 32768 sharded 8 ways (4096 rows/core);
    weights replicated.
  - The conv is linear, so it folds into the first Linear layer:
        conv(x).flat @ W1 == x @ (C @ W1)
    where C[784,676] is the conv-as-matrix. The host computes
    W1p = C @ W1 directly and pre-transposes x to pixel-major
    [784, 4096] per core, so the device DMAs matmul-ready [pix, batch]
    tiles and the PE spends zero cycles transposing inputs.
  - All matmul operands are bf16 (full-rate PE path, half the
    stationary-load time and DMA bytes of fp32r); PSUM accumulation
    stays fp32, biases applied in fp32 on ScalarE during PSUM eviction.
    End-to-end bf16 error ~5e-3, well under the 2e-2 gate.
  - Pixel dim is chunked 7x112 (not 6x128+16) so each batch-tile group
    loads with ONE large DMA ([112 part, 7*cols] rearranged view) —
    per-DMA fixed cost dominated the v2 profile. Input group DMAs
    alternate between the two HWDGE queues (sync + scalar engines);
    all loads are pre-issued at build time.
  - Per 512-row batch tile: three chained matmul layers with features
    on partitions / batch in the free dim; final [10,512] tile
    PE-transposed back to batch-major (10-col moving streams: ~free).
  - Last two tiles split into 256-col halves to pipeline the drain.
"""

import sys

for _p in (
    "/opt/trn_rl_repo",
    "/root/.axon_site",
    "/root/.axon_site/_ro/trn_rl_repo",
    "/root/.axon_site/_ro/pypackages",
):
    if _p not in sys.path:
        sys.path.append(_p)

from contextlib import ExitStack

import numpy as np
import ml_dtypes

import concourse.bass as bass
import concourse.tile as tile
from concourse import mybir
from concourse.bass_utils import run_bass_kernel_spmd
from concourse.masks import make_identity

F32 = mybir.dt.float32
BF16 = mybir.dt.bfloat16
AFT = mybir.ActivationFunctionType
NP_BF16 = ml_dtypes.bfloat16

B_FULL = 32768
N_CORES = 8
B_CORE = B_FULL // N_CORES  # 4096
IMG = 28
OHW = 26
FLAT = OHW * OHW  # 676
NPIX = IMG * IMG  # 784
HID = 300
NCLS = 10

BT = 512  # batch tile (matmul moving free dim; PSUM bank = 512 fp32)
NBT = B_CORE // BT  # 8
NBC = BT // 128  # 4 x 128-row chunks per batch tile

PW = 112  # pixel chunk: 784 = 7*112, uniform -> single-DMA group loads
NPC = NPIX // PW  # 7
H_CH = [(s, min(128, HID - s)) for s in range(0, HID, 128)]  # 3 chunks

# batch-tile groups, each loaded by one DMA: [1, 2, 2, 2, 1] tiles
GROUPS = [(0, 1), (1, 2), (3, 2), (5, 2), (7, 1)]


def _legalize_single_wait(nc):
    """This walrus build accepts only one sync-wait per instruction; move
    extra waits onto NoOps inserted just before, on the same engine."""
    n = 0
    for fn in nc.m.functions:
        for bb in fn.blocks:
            new_insts = []
            for inst in bb.instructions:
                si = inst.sync_info
                if si is not None and si.on_wait and len(si.on_wait) > 1:
                    waits = list(si.on_wait)
                    for w in waits[:-1]:
                        nop = mybir.InstNoOp(
                            name=f"{inst.name}-w{n}",
                            sync_info=mybir.SyncInfo(on_wait=[w], on_update=[]),
                            bass_nofuse=True,
                            engine=inst.engine,
                        )
                        n += 1
                        nc.register_instruction(nop, overwrite=True)
                        new_insts.append(nop)
                    inst.sync_info = mybir.SyncInfo(
                        on_wait=[waits[-1]], on_update=list(si.on_update)
                    )
                new_insts.append(inst)
            bb.instructions = new_insts
    return n


def _emit(ctx: ExitStack, tc: tile.TileContext, xt_d, w1p, b1, w2, b2, w3, b3, out):
    nc = tc.nc

    const = ctx.enter_context(tc.tile_pool(name="const", bufs=1))
    psmm = ctx.enter_context(tc.tile_pool(name="psmm", bufs=6, space="PSUM"))
    pso = ctx.enter_context(tc.tile_pool(name="pso", bufs=2, space="PSUM"))
    hp_ = ctx.enter_context(tc.tile_pool(name="hp", bufs=2))
    op_ = ctx.enter_context(tc.tile_pool(name="op", bufs=2))
    obp = ctx.enter_context(tc.tile_pool(name="obp", bufs=8))

    ident = const.tile([128, 128], F32, name="ident")
    make_identity(nc, ident)
    identb = const.tile([128, 128], BF16, name="identb")
    nc.vector.tensor_copy(identb[:, :], ident[:, :])

    # --- all input DMAs pre-issued, split across the two HWDGE queues ---
    # x.T view [PW, NPC, B] so one DMA loads all 7 pixel chunks of a group
    xt_v = xt_d.rearrange("(c p) b -> p c b", c=NPC)
    xg = []
    for gi, (t0, nt) in enumerate(GROUPS):
        g = const.tile([PW, NPC, nt * BT], BF16, name=f"xg{gi}")
        eng = nc.sync if gi % 2 == 0 else nc.scalar
        eng.dma_start(g[:, :, :], xt_v[:, :, t0 * BT : (t0 + nt) * BT])
        xg.append(g)
    # weights (scalar queue): w1p as one DMA via the same 112-chunk trick
    w1pt = const.tile([PW, NPC, HID], BF16, name="w1pt")
    nc.scalar.dma_start(
        w1pt[:, :, :], w1p.rearrange("(c p) h -> p c h", c=NPC)
    )
    w1ps = [w1pt[:, pc, :] for pc in range(NPC)]
    b1s, b2s, w2s, w3s = [], [], [], []
    for hc, (h0, hp) in enumerate(H_CH):
        bt1 = const.tile([hp, 1], F32, name=f"b1s{hc}")
        nc.scalar.dma_start(bt1[:, :], b1[h0 : h0 + hp, :])
        b1s.append(bt1)
        wt2 = const.tile([hp, HID], BF16, name=f"w2s{hc}")
        nc.scalar.dma_start(wt2[:, :], w2[h0 : h0 + hp, :])
        w2s.append(wt2)
        bt2 = const.tile([hp, 1], F32, name=f"b2s{hc}")
        nc.scalar.dma_start(bt2[:, :], b2[h0 : h0 + hp, :])
        b2s.append(bt2)
        wt3 = const.tile([hp, NCLS], BF16, name=f"w3s{hc}")
        nc.scalar.dma_start(wt3[:, :], w3[h0 : h0 + hp, :])
        w3s.append(wt3)
    b3s = const.tile([NCLS, 1], F32, name="b3s")
    nc.scalar.dma_start(b3s[:, :], b3[:, :])

    # Dense PE warmup burst: keeps the HAM clock gate ramping while the
    # first input group + weights DMA in (~5.5us).
    warm = psmm.tile([128, BT], F32, name="warm", tag="psf")
    for _ in range(50):
        nc.tensor.matmul(
            warm[0:128, 0:128], identb[:, :], identb[:, :],
            start=True, stop=True,
        )

    # tile index -> (group tile view, col offset of tile in group)
    tview = {}
    for gi, (t0, nt) in enumerate(GROUPS):
        for k in range(nt):
            tview[t0 + k] = (xg[gi], k * BT)

    def compute(t, off, n):
        """fc1->fc2->fc3->store for batch columns [off, off+n) of tile t."""
        g, goff = tview[t]
        c0 = goff + off
        # fc1: relu(x @ W1p + b1), output hidden-major [300, n]
        h1 = []
        for hc, (h0, hp) in enumerate(H_CH):
            ps = psmm.tile([128, BT], F32, name="ps1", tag="psf")
            for pc in range(NPC):
                nc.tensor.matmul(
                    ps[0:hp, 0:n],
                    w1ps[pc][0:PW, h0 : h0 + hp],
                    g[0:PW, pc, c0 : c0 + n],
                    start=(pc == 0),
                    stop=(pc == NPC - 1),
                )
            h = hp_.tile([hp, BT], BF16, name=f"h1_{hc}", tag=f"h1_{hc}")
            nc.scalar.activation(
                h[:, 0:n], ps[0:hp, 0:n], AFT.Relu, bias=b1s[hc][:, :]
            )
            h1.append(h)

        # fc2: relu(h1 @ W2 + b2) — k-outer so all m-groups unblock on h1[0]
        ps2 = [
            psmm.tile([128, BT], F32, name=f"ps2_{hc2}", tag="psf")
            for hc2 in range(len(H_CH))
        ]
        for hc, (h0, hp) in enumerate(H_CH):
            for hc2, (g0, gp) in enumerate(H_CH):
                nc.tensor.matmul(
                    ps2[hc2][0:gp, 0:n],
                    w2s[hc][0:hp, g0 : g0 + gp],
                    h1[hc][0:hp, 0:n],
                    start=(hc == 0),
                    stop=(hc == len(H_CH) - 1),
                )
        h2 = []
        for hc2, (g0, gp) in enumerate(H_CH):
            h = hp_.tile([gp, BT], BF16, name=f"h2_{hc2}", tag=f"h2_{hc2}")
            nc.scalar.activation(
                h[:, 0:n], ps2[hc2][0:gp, 0:n], AFT.Relu, bias=b2s[hc2][:, :]
            )
            h2.append(h)

        # fc3: h2 @ W3 + b3 -> [10, n]
        ps = psmm.tile([128, BT], F32, name="ps3", tag="psf")
        for hc, (h0, hp) in enumerate(H_CH):
            nc.tensor.matmul(
                ps[0:NCLS, 0:n],
                w3s[hc][0:hp, 0:NCLS],
                h2[hc][0:hp, 0:n],
                start=(hc == 0),
                stop=(hc == len(H_CH) - 1),
            )
        ob = op_.tile([NCLS, BT], BF16, name="ob", tag="ob")
        nc.scalar.activation(
            ob[:, 0:n], ps[0:NCLS, 0:n], AFT.Identity, bias=b3s[:, :]
        )

        # transpose [10, n] back to batch-major (10-col moving: ~free) and
        # store; the bf16 round of the final logits costs ~0.2% extra error
        nbc = n // 128
        po = pso.tile([128, NBC * NCLS], BF16, name="po", tag="po")
        for bc in range(nbc):
            nc.tensor.transpose(
                po[0:128, bc * NCLS : (bc + 1) * NCLS],
                ob[:, bc * 128 : (bc + 1) * 128],
                identb[0:NCLS, 0:NCLS],
            )
        os_ = obp.tile([128, NBC * NCLS], F32, name="os", tag="os")
        nc.vector.tensor_copy(os_[:, 0 : nbc * NCLS], po[0:128, 0 : nbc * NCLS])
        r0 = t * BT
        nc.sync.dma_start(
            out[r0 + off : r0 + off + n, :].rearrange("(bc b) c -> b bc c", bc=nbc),
            os_[:, 0 : nbc * NCLS].rearrange("b (bc c) -> b bc c", bc=nbc),
        )

    for t in range(NBT):
        if t >= NBT - 2:
            # split the last two tiles to pipeline the serial drain chain
            compute(t, 0, 256)
            compute(t, 256, 256)
        else:
            compute(t, 0, BT)


def _build_c(conv_w: np.ndarray) -> np.ndarray:
    """C[p, q] with conv(x).flat = x @ C. Pure scatter of conv_w."""
    c = np.zeros((NPIX, FLAT), np.float32)
    oi = np.arange(OHW)
    oj = np.arange(OHW)
    q = (oi[:, None] * OHW + oj[None, :]).ravel()
    for dy in range(3):
        for dx in range(3):
            p = ((oi[:, None] + dy) * IMG + (oj[None, :] + dx)).ravel()
            c[p, q] = conv_w[dy, dx]
    return c


_NC_CACHE: list = []


def _get_nc():
    if _NC_CACHE:
        return _NC_CACHE[0]
    nc = bass.Bass("TRN2", target_bir_lowering=False, debug=False)
    xt_d = nc.dram_tensor("xt", [NPIX, B_CORE], BF16, kind="ExternalInput").ap()
    w1p = nc.dram_tensor("w1p", [NPIX, HID], BF16, kind="ExternalInput").ap()
    b1 = nc.dram_tensor("b1", [HID, 1], F32, kind="ExternalInput").ap()
    w2 = nc.dram_tensor("w2", [HID, HID], BF16, kind="ExternalInput").ap()
    b2 = nc.dram_tensor("b2", [HID, 1], F32, kind="ExternalInput").ap()
    w3 = nc.dram_tensor("w3", [HID, NCLS], BF16, kind="ExternalInput").ap()
    b3 = nc.dram_tensor("b3", [NCLS, 1], F32, kind="ExternalInput").ap()
    out = nc.dram_tensor("out", [B_CORE, NCLS], F32, kind="ExternalOutput").ap()
    with tile.TileContext(nc) as tc:
        with ExitStack() as ctx:
            _emit(ctx, tc, xt_d, w1p, b1, w2, b2, w3, b3, out)
    _legalize_single_wait(nc)
    _NC_CACHE.append(nc)
    return nc


def _in_maps(inputs: dict) -> list:
    x = np.asarray(inputs["x"], dtype=np.float32)
    assert x.shape == (B_FULL, NPIX), x.shape
    c = _build_c(np.asarray(inputs["conv_w"], dtype=np.float32))
    w1p = (c @ np.asarray(inputs["W1"], np.float32)).astype(NP_BF16)
    xb = x.astype(NP_BF16)
    common = {
        "w1p": np.ascontiguousarray(w1p),
        "b1": np.asarray(inputs["b1"], np.float32).reshape(HID, 1),
        "w2": np.asarray(inputs["W2"], np.float32).astype(NP_BF16),
        "b2": np.asarray(inputs["b2"], np.float32).reshape(HID, 1),
        "w3": np.asarray(inputs["W3"], np.float32).astype(NP_BF16),
        "b3": np.asarray(inputs["b3"], np.float32).reshape(NCLS, 1),
    }
    return [
        {
            "xt": np.ascontiguousarray(xb[c_ * B_CORE : (c_ + 1) * B_CORE].T),
            **common,
        }
        for c_ in range(N_CORES)
    ]


def kernel(**inputs) -> np.ndarray:
    nc = _get_nc()
    res = run_bass_kernel_spmd(nc, _in_maps(inputs), list(range(N_CORES)))
    return np.concatenate(
        [res.results[c]["out"] for c in range(N_CORES)], axis=0
    )


if __name__ == "__main__":
    rng = np.random.default_rng(0)
    ins = {
        "x": rng.standard_normal((B_FULL, NPIX), dtype=np.float32),
        "conv_w": rng.standard_normal((3, 3), dtype=np.float32) * 0.1,
        "W1": rng.standard_normal((FLAT, HID), dtype=np.float32) * 0.04,
        "b1": np.zeros(HID, np.float32),
        "W2": rng.standard_normal((HID, HID), dtype=np.float32) * 0.06,
        "b2": np.zeros(HID, np.float32),
        "W3": rng.standard_normal((HID, NCLS), dtype=np.float32) * 0.06,
        "b3": np.zeros(NCLS, np.float32),
    }
    y = kernel(**ins)
    # numpy reference with explicit conv
    from numpy.lib.stride_tricks import sliding_window_view

    img = ins["x"].reshape(-1, IMG, IMG)
    win = sliding_window_view(img, (3, 3), axis=(1, 2))
    conv = np.einsum("bijkl,kl->bij", win, ins["conv_w"]).reshape(-1, FLAT)
    h = np.maximum(conv @ ins["W1"] + ins["b1"], 0)
    h = np.maximum(h @ ins["W2"] + ins["b2"], 0)
    ref = h @ ins["W3"] + ins["b3"]
    err = np.abs(y - ref).max() / (np.abs(ref).max() + 1e-9)
    print("max rel err vs numpy:", err)
